# revision 22
# baseline (speedup 1.0000x reference)
"""Trainium2 Bass kernel for nn_KernelToeplitzCausalLinear.

Computes, for x (B=8, E=2048, S=1024), weight (4, 1024), bias (1024,):

    out[b, e, t] = sum_k sum_{s<=t} x[b, e+k-3, s] * weight[k, t-s] + bias[t]

i.e. a causal 4-tap shift along E combined with a full causal (upper-
triangular Toeplitz) matmul along the dim axis.

Sharding: data-parallel over batch B -> one NeuronCore per batch element
(no halo: the E-shifts stay within a batch element).

v3 design (bf16 datapath, tol 2e-2 >> bf16's ~2e-3; K-stacked taps):
  * The 4 taps are folded into the matmul CONTRACTION: each 128-deep
    contraction block covers (4 taps x 32 s-values), so one streamed
    column does all taps at once and the causal triangle is chunked at
    32-column granularity -- 270336 streamed columns/core vs 294912 for
    tap-separate 128-granular chunks (the exact stream floor for K=128).
  * Host pre-builds x4[c, e, 32k+s'] = x[e+k-3, 32c+s'] (bf16, zero pad
    for e+k-3<0) so the DMA XBAR transpose loads the stacked strips
    XTS[c] (128 x 2048) directly -- no PE transposes, no on-chip
    replication.  Weight blocks collapse to ONE strip
    W0[32k+s', d] = weight[k, d-s'] (d>=s' else 0): the chunk-c block is
    just W0[:, c0-32c : c1-32c].
  * Per 128-row e-tile j: 48 bf16 matmuls (32 c-chunks, bank-split)
    accumulate into a 2-bank PSUM pair; bias is added during the
    PSUM->SBUF copy (DVE); fp32 out DMA (issued from DVE's DGE so the
    SP/Act DGEs only carry the strip transposes and reps pipeline).
"""
import numpy as np
from contextlib import ExitStack

import ml_dtypes

import concourse.bass as bass
import concourse.tile as tile
from concourse import bacc, mybir
from concourse.bass_utils import run_bass_kernel_spmd

P = 128
B = 8
E = 2048
S = 1024
K = 4
NC = 32              # c-chunks of 32 s-values
NJ = E // P          # 16 e-tiles
ECH = 1024           # e-rows per transposing DMA chunk
F32 = mybir.dt.float32
BF16 = mybir.dt.bfloat16
PSUM_BUFS = 8


def _chunks(c):
    """Output-column chunks for stacked block c (s in [32c, 32c+32)):
    columns t >= 32c, split at the 512 PSUM bank boundary."""
    c0 = 32 * c
    if c0 < 512:
        return [(c0, 512), (512, 1024)]
    return [(c0, 1024)]


def make_w0(weight: np.ndarray) -> np.ndarray:
    """(4, 1024) -> (128, 1024) stacked strip W0[32k+s', d] = w[k, d-s']
    for d >= s' else 0 (bf16)."""
    sp = np.arange(32)
    d = np.arange(S)
    w0 = np.zeros((P, S), np.float32)
    for k in range(K):
        idx = d[None, :] - sp[:, None]          # (32, 1024)
        valid = idx >= 0
        w0[32 * k: 32 * k + 32] = np.where(
            valid, weight[k, idx.clip(0, S - 1)], 0.0)
    return np.ascontiguousarray(w0.astype(ml_dtypes.bfloat16))


def make_x4(x: np.ndarray) -> np.ndarray:
    """(B, E, S) fp32 -> (B, 32, 128, E) bf16 with
    x4[b, c, 32k+s', e] = x[b, e+k-3, 32c+s'] (0 when e+k-3 < 0).
    Pre-transposed on host so the device loads strips with plain
    contiguous DMA (2KB runs), no XBAR."""
    xb = x.astype(ml_dtypes.bfloat16)
    xp = np.zeros((B, E + 3, S), ml_dtypes.bfloat16)
    xp[:, 3:] = xb
    x4 = np.empty((B, NC, P, E), ml_dtypes.bfloat16)
    for k in range(K):
        a = xp[:, k:k + E, :].transpose(0, 2, 1).reshape(B, NC, 32, E)
        x4[:, :, 32 * k: 32 * k + 32, :] = a
    return np.ascontiguousarray(x4)


def build_nc(reps: int = 1):
    nc = bacc.Bacc("TRN2", target_bir_lowering=False, debug=False)
    x_d = nc.dram_tensor("x4", [NC, P, E], BF16, kind="ExternalInput").ap()
    w_d = nc.dram_tensor("w0", [P, S], BF16, kind="ExternalInput").ap()
    b_d = nc.dram_tensor("bias", [P, S], F32, kind="ExternalInput").ap()
    o_d = nc.dram_tensor("out", [E, S], F32, kind="ExternalOutput").ap()

    with tile.TileContext(nc) as tc, ExitStack() as ctx:
        consts = ctx.enter_context(tc.tile_pool(name="consts", bufs=1))
        xt_pool = ctx.enter_context(tc.tile_pool(name="xt", bufs=1))
        osb_pool = ctx.enter_context(tc.tile_pool(name="osb", bufs=3))
        opsum = ctx.enter_context(tc.tile_pool(name="opsum", bufs=PSUM_BUFS,
                                               space="PSUM"))

        bias_rep = consts.tile([P, S], F32)
        nc.sync.dma_start(bias_rep[:], b_d[:])
        W0 = consts.tile([P, S], BF16)
        nc.sync.dma_start(W0[:], w_d[:])

        XTS = [xt_pool.tile([P, E], BF16, name=f"xts{c}") for c in range(NC)]

        def body(_iv=None):
            # stacked x.T strips: plain contiguous DMA (host pre-transposed)
            for m in range(E // ECH):
                for c in range(NC):
                    nc.sync.dma_start(
                        XTS[c][:, m * ECH:(m + 1) * ECH],
                        x_d[c, :, m * ECH:(m + 1) * ECH],
                    )

            for j in range(NJ):
                pts = [opsum.tile([P, 512], F32, name="ob") for _ in range(2)]
                mms = []
                for c in range(NC):
                    lhsT = XTS[c][:, j * P:(j + 1) * P]
                    for (c0, c1) in _chunks(c):
                        bank = 1 if c0 >= 512 else 0
                        rhs = w0_slice = W0[:, c0 - 32 * c: c1 - 32 * c]
                        outap = pts[bank][:, c0 - 512 * bank: c1 - 512 * bank]
                        mms.append((bank, outap, lhsT, rhs))
                seen = set()
                last_idx = {b: max(i for i, m in enumerate(mms) if m[0] == b)
                            for b in (0, 1)}
                for i, (bank, outap, lhsT, rhs) in enumerate(mms):
                    nc.tensor.matmul(
                        outap, lhsT, rhs,
                        start=bank not in seen,
                        stop=i == last_idx[bank],
                    )
                    seen.add(bank)

                osb = osb_pool.tile([P, S], F32, name="osb")
                for h in range(2):
                    nc.vector.tensor_add(
                        osb[:, h * 512:(h + 1) * 512], pts[h][:],
                        bias_rep[:, h * 512:(h + 1) * 512],
                    )
                nc.scalar.dma_start(o_d[j * P:(j + 1) * P, :], osb[:])

        if reps == 1:
            body()
        else:
            with tc.For_i(0, reps, 1):
                body()

    nc.compile()
    return nc


def make_inmaps(x: np.ndarray, weight: np.ndarray, bias: np.ndarray):
    x = np.asarray(x, dtype=np.float32)
    weight = np.asarray(weight, dtype=np.float32)
    bias = np.asarray(bias, dtype=np.float32)
    assert x.shape == (B, E, S), x.shape
    assert weight.shape == (K, S), weight.shape
    assert bias.shape == (S,), bias.shape
    w0 = make_w0(weight)
    x4 = make_x4(x)
    bias_rep = np.ascontiguousarray(
        np.broadcast_to(bias, (P, S)).astype(np.float32))
    return [
        {"x4": x4[b], "w0": w0, "bias": bias_rep}
        for b in range(B)
    ]


_NC_CACHE = {}


def _get_nc():
    if 'nc' not in _NC_CACHE:
        _NC_CACHE['nc'] = build_nc(1)
    return _NC_CACHE['nc']


def kernel(x: np.ndarray, weight: np.ndarray, bias: np.ndarray) -> np.ndarray:
    in_maps = make_inmaps(x, weight, bias)
    nc = _get_nc()
    res = run_bass_kernel_spmd(nc, in_maps, list(range(B)))
    out = np.stack([res.results[b]["out"] for b in range(B)]).astype(np.float32)
    return out


# revision 23
# speedup vs baseline: 1.0656x; 1.0656x over previous
"""Trainium2 Bass kernel for nn_KernelToeplitzCausalLinear.

Computes, for x (B=8, E=2048, S=1024), weight (4, 1024), bias (1024,):

    out[b, e, t] = sum_k sum_{s<=t} x[b, e+k-3, s] * weight[k, t-s] + bias[t]

i.e. a causal 4-tap shift along E combined with a full causal (upper-
triangular Toeplitz) matmul along the dim axis.

Sharding: data-parallel over batch B -> one NeuronCore per batch element
(no halo: the E-shifts stay within a batch element).

v3 design (bf16 datapath, tol 2e-2 >> bf16's ~2e-3; K-stacked taps):
  * The 4 taps are folded into the matmul CONTRACTION: each 128-deep
    contraction block covers (4 taps x 32 s-values), so one streamed
    column does all taps at once and the causal triangle is chunked at
    32-column granularity -- 270336 streamed columns/core vs 294912 for
    tap-separate 128-granular chunks (the exact stream floor for K=128).
  * Host pre-builds x4[c, e, 32k+s'] = x[e+k-3, 32c+s'] (bf16, zero pad
    for e+k-3<0) so the DMA XBAR transpose loads the stacked strips
    XTS[c] (128 x 2048) directly -- no PE transposes, no on-chip
    replication.  Weight blocks collapse to ONE strip
    W0[32k+s', d] = weight[k, d-s'] (d>=s' else 0): the chunk-c block is
    just W0[:, c0-32c : c1-32c].
  * Per 128-row e-tile j: 48 bf16 matmuls (32 c-chunks, bank-split)
    accumulate into a 2-bank PSUM pair; bias is added during the
    PSUM->SBUF copy (DVE); fp32 out DMA (issued from DVE's DGE so the
    SP/Act DGEs only carry the strip transposes and reps pipeline).
"""
import numpy as np
from contextlib import ExitStack

import ml_dtypes

import concourse.bass as bass
import concourse.tile as tile
from concourse import bacc, mybir
from concourse.bass_utils import run_bass_kernel_spmd

P = 128
B = 8
E = 2048
S = 1024
K = 4
NC = 32              # c-chunks of 32 s-values
NJ = E // P          # 16 e-tiles
ECH = 1024           # e-rows per transposing DMA chunk
F32 = mybir.dt.float32
BF16 = mybir.dt.bfloat16
PSUM_BUFS = 6


def _chunks(c):
    """Output-column chunks for stacked block c (s in [32c, 32c+32)):
    columns t >= 32c, split at the 512 PSUM bank boundary."""
    c0 = 32 * c
    if c0 < 512:
        return [(c0, 512), (512, 1024)]
    return [(c0, 1024)]


def make_w0(weight: np.ndarray) -> np.ndarray:
    """(4, 1024) -> (128, 1024) stacked strip W0[32k+s', d] = w[k, d-s']
    for d >= s' else 0 (bf16)."""
    sp = np.arange(32)
    d = np.arange(S)
    w0 = np.zeros((P, S), np.float32)
    for k in range(K):
        idx = d[None, :] - sp[:, None]          # (32, 1024)
        valid = idx >= 0
        w0[32 * k: 32 * k + 32] = np.where(
            valid, weight[k, idx.clip(0, S - 1)], 0.0)
    return np.ascontiguousarray(w0.astype(ml_dtypes.bfloat16))


def make_x4(x: np.ndarray) -> np.ndarray:
    """(B, E, S) fp32 -> (B, 32, 128, E) bf16 with
    x4[b, c, 32k+s', e] = x[b, e+k-3, 32c+s'] (0 when e+k-3 < 0).
    Pre-transposed on host so the device loads strips with plain
    contiguous DMA (2KB runs), no XBAR."""
    xb = x.astype(ml_dtypes.bfloat16)
    xp = np.zeros((B, E + 3, S), ml_dtypes.bfloat16)
    xp[:, 3:] = xb
    x4 = np.empty((B, NC, P, E), ml_dtypes.bfloat16)
    for k in range(K):
        a = xp[:, k:k + E, :].transpose(0, 2, 1).reshape(B, NC, 32, E)
        x4[:, :, 32 * k: 32 * k + 32, :] = a
    return np.ascontiguousarray(x4)


def build_nc(reps: int = 1):
    nc = bacc.Bacc("TRN2", target_bir_lowering=False, debug=False)
    x_d = nc.dram_tensor("x4", [NC, P, E], BF16, kind="ExternalInput").ap()
    w_d = nc.dram_tensor("w0", [P, S], BF16, kind="ExternalInput").ap()
    b_d = nc.dram_tensor("bias", [P, S], F32, kind="ExternalInput").ap()
    o_d = nc.dram_tensor("out", [E, S], F32, kind="ExternalOutput").ap()

    with tile.TileContext(nc) as tc, ExitStack() as ctx:
        consts = ctx.enter_context(tc.tile_pool(name="consts", bufs=1))
        xt_pool = ctx.enter_context(tc.tile_pool(name="xt", bufs=1))
        osb_pool = ctx.enter_context(tc.tile_pool(name="osb", bufs=4))
        opsum = ctx.enter_context(tc.tile_pool(name="opsum", bufs=PSUM_BUFS,
                                               space="PSUM"))

        bias_rep = consts.tile([P, S], F32)
        nc.sync.dma_start(bias_rep[:], b_d[:])
        W0 = consts.tile([P, S], BF16)
        nc.sync.dma_start(W0[:], w_d[:])

        XTS = [xt_pool.tile([P, E], BF16, name=f"xts{c}") for c in range(NC)]

        def body(_iv=None):
            # stacked x.T strips: plain contiguous DMA (host pre-transposed)
            for m in range(E // ECH):
                for c in range(NC):
                    nc.sync.dma_start(
                        XTS[c][:, m * ECH:(m + 1) * ECH],
                        x_d[c, :, m * ECH:(m + 1) * ECH],
                    )

            for j in range(NJ):
                pts = [opsum.tile([P, 512], F32, name="ob") for _ in range(2)]
                mms = []
                for c in range(NC):
                    lhsT = XTS[c][:, j * P:(j + 1) * P]
                    for (c0, c1) in _chunks(c):
                        bank = 1 if c0 >= 512 else 0
                        rhs = w0_slice = W0[:, c0 - 32 * c: c1 - 32 * c]
                        outap = pts[bank][:, c0 - 512 * bank: c1 - 512 * bank]
                        mms.append((bank, outap, lhsT, rhs))
                seen = set()
                last_idx = {b: max(i for i, m in enumerate(mms) if m[0] == b)
                            for b in (0, 1)}
                for i, (bank, outap, lhsT, rhs) in enumerate(mms):
                    nc.tensor.matmul(
                        outap, lhsT, rhs,
                        start=bank not in seen,
                        stop=i == last_idx[bank],
                    )
                    seen.add(bank)

                osb = osb_pool.tile([P, S], F32, name="osb")
                for h in range(2):
                    nc.vector.tensor_add(
                        osb[:, h * 512:(h + 1) * 512], pts[h][:],
                        bias_rep[:, h * 512:(h + 1) * 512],
                    )
                nc.scalar.dma_start(o_d[j * P:(j + 1) * P, :], osb[:])

        if reps == 1:
            body()
        else:
            with tc.For_i(0, reps, 1):
                body()

    nc.compile()
    return nc


def make_inmaps(x: np.ndarray, weight: np.ndarray, bias: np.ndarray):
    x = np.asarray(x, dtype=np.float32)
    weight = np.asarray(weight, dtype=np.float32)
    bias = np.asarray(bias, dtype=np.float32)
    assert x.shape == (B, E, S), x.shape
    assert weight.shape == (K, S), weight.shape
    assert bias.shape == (S,), bias.shape
    w0 = make_w0(weight)
    x4 = make_x4(x)
    bias_rep = np.ascontiguousarray(
        np.broadcast_to(bias, (P, S)).astype(np.float32))
    return [
        {"x4": x4[b], "w0": w0, "bias": bias_rep}
        for b in range(B)
    ]


_NC_CACHE = {}


def _get_nc():
    if 'nc' not in _NC_CACHE:
        _NC_CACHE['nc'] = build_nc(1)
    return _NC_CACHE['nc']


def kernel(x: np.ndarray, weight: np.ndarray, bias: np.ndarray) -> np.ndarray:
    in_maps = make_inmaps(x, weight, bias)
    nc = _get_nc()
    res = run_bass_kernel_spmd(nc, in_maps, list(range(B)))
    out = np.stack([res.results[b]["out"] for b in range(B)]).astype(np.float32)
    return out
